# revision 38
# baseline (speedup 1.0000x reference)
"""Multi-head attention Trainium2 kernel.

B=8, S=1024, D=1024, H=16 heads, head_dim=64.
Sharding: pure data parallelism over batch — one batch element per
NeuronCore, weights replicated, no collectives.

Per-core dataflow (all matmul operands bf16, fp32 PSUM accumulate):
  host:   xT = x.T (k-major) for q/k/v, WT = W.T for all weights, bf16;
          bias rows pre-broadcast to [128, .] host-side.
  PSUM = two 2-slot FIFOs of [128,1024]f32 tiles (psS: scores/waves/tail
  outproj, psC: Q chains + AV + in-sweep outproj) = all 8 banks.
  DMA order V, Q, K (K reuses V's SBUF slots): V/K projection chains are
  emitted k-major in software-pipelined waves of 4 so the PE consumes x/w
  tiles at the DMA arrival rate; Q projection chains, AV chains, normalize
  finishers and outproj chunks are split into 4-matmul filler groups
  interleaved between score j-tiles, keeping the PE fed while ACT drains
  the exp stream.
  Per (head-pair p, q-chunk c):
    scores j-tile: [128,1024] psum, E heads rows 0-63 -> cols 0:512,
    O heads -> cols 512:1024; one ACT exp drains both (bf16 SBUF).
    AV: one [128,1024] psum tile: E chain cols 0:512 (rows 0-63 OT_even,
    row 64 colsum_e), O chain cols 512:1024 (rows 64-127 OT_odd, row 32
    colsum_o).  Normalize: DVE stages denom rows + recip_approx_fast;
    a deferred finisher broadcasts the two recip rows across partitions
    with 0-stride-AP DMAs (rep_row), then same-lane DVE muls -> OT.
    Q/K projection bias-adds run on ACT (Identity + per-partition bias,
    same act table as Exp) to keep the DVE queue off the critical path.
  out[s,do] = sum_dv OT[dv,s]*WoT[dv,do] (+bo) -> bf16 -> DRAM.
"""

import numpy as np
import ml_dtypes
from contextlib import ExitStack

import concourse.bass as bass
import concourse.tile as tile
import concourse.mybir as mybir
from concourse import bacc
from concourse.bass_utils import run_bass_kernel_spmd

BF16 = mybir.dt.bfloat16
F32 = mybir.dt.float32
AF = mybir.ActivationFunctionType

S = 1024
D = 1024
H = 16
HD = 64
P = 128
KT = D // P      # 8 contraction tiles
MT = S // P      # 8 row tiles
NC = 512         # matmul moving-dim chunk
NCH = S // NC    # 2 chunks
PAIRS = H // 2   # 8
VW = 160  # per-pair V65 width [V_e(64) | ones@64 | gap 65-95 | V_o@96-159]
N_CORES = 8


def rep_row(row_ap, n):
    """[1, N] AP -> [1, n, N] view with a 0-stride middle dim: DMA reads the
    row n times, writing n partitions (HW-validated row broadcast)."""
    (pstep, pnum), (estep, enum) = row_ap.ap[0], row_ap.ap[1]
    assert pnum == 1
    return bass.AP(row_ap.tensor, row_ap.offset, [[pstep, 1], [0, n], [estep, enum]])


def build_body(ctx: ExitStack, tc, io):
    nc = tc.nc

    const = ctx.enter_context(tc.tile_pool(name="const", bufs=1))
    qkt = ctx.enter_context(tc.tile_pool(name="qkt", bufs=1))
    v65p = ctx.enter_context(tc.tile_pool(name="v65", bufs=1))
    otp = ctx.enter_context(tc.tile_pool(name="otp", bufs=1))
    xwA = ctx.enter_context(tc.tile_pool(name="xwA", bufs=1))   # V then K
    xwQ = ctx.enter_context(tc.tile_pool(name="xwQ", bufs=1))   # Q
    wop = ctx.enter_context(tc.tile_pool(name="wop", bufs=1))
    expp = ctx.enter_context(tc.tile_pool(name="expp", bufs=2))
    nrm = ctx.enter_context(tc.tile_pool(name="nrm", bufs=2))
    osbp = ctx.enter_context(tc.tile_pool(name="osb", bufs=2))
    # PSUM split: scores FIFO (2 slots) + chain FIFO (2 slots), 2 banks each
    psS = ctx.enter_context(tc.tile_pool(name="psS", bufs=2, space="PSUM"))
    psC = ctx.enter_context(tc.tile_pool(name="psC", bufs=2, space="PSUM"))

    def s_tile(name):
        return psS.tile([P, 2 * NC], F32, tag="s", name=name)

    def c_tile(name):
        return psC.tile([P, 2 * NC], F32, tag="c", name=name)

    # ---- PE warm-up: dummy matmuls keep the PE busy from t=0 while the
    # first x/w DMAs land, so real chains start at a warm clock.
    warm = const.tile([P, NC], BF16, tag="warm")
    nc.vector.memset(warm[:], 0.0)
    for i in range(3):
        wps = s_tile("warmps")
        nc.tensor.matmul(wps[:, 0:NC], warm[:, 0:P], warm[:], start=True, stop=True)
        nc.tensor.matmul(wps[:, NC:], warm[:, 0:P], warm[:], start=True, stop=True)

    # ---- constants (bias tensors arrive pre-broadcast from the host) ----
    bqc = const.tile([P, KT], F32, tag="bqc")
    nc.sync.dma_start(bqc[:], io["bq_c"][:])
    bkc = const.tile([P, KT], F32, tag="bkc")
    nc.sync.dma_start(bkc[:], io["bk_c"][:])

    # ---- persistent activation tiles ----
    QT = [qkt.tile([P, S], BF16, tag=f"qt{m}", name=f"qt{m}") for m in range(MT)]
    KTt = [qkt.tile([P, S], BF16, tag=f"kt{m}", name=f"ktt{m}") for m in range(MT)]
    V65 = [v65p.tile([P, PAIRS * VW], BF16, tag=f"v65_{m}", name=f"v65_{m}") for m in range(MT)]
    OT = [otp.tile([P, S], BF16, tag=f"ot{m}", name=f"ot{m}") for m in range(MT)]

    # ones column at col 64 of each 160-wide pair block; zero the gap
    for m in range(MT):
        v = V65[m].rearrange("p (pr w) -> p pr w", w=VW)
        nc.vector.memset(v[:, :, HD : HD + 1], 1.0)
        nc.vector.memset(v[:, :, HD + 1 : 96], 0.0)

    def load_xw(pool, xdram, wdram, pfx):
        xt = [pool.tile([P, S], BF16, tag=f"{pfx}x{k}", name=f"{pfx}xt{k}")
              for k in range(KT)]
        wt = [pool.tile([P, D], BF16, tag=f"{pfx}w{k}", name=f"{pfx}wt{k}")
              for k in range(KT)]
        for k in range(KT):
            nc.sync.dma_start(xt[k][:], xdram[k * P : (k + 1) * P, :])
            nc.sync.dma_start(wt[k][:], wdram[k * P : (k + 1) * P, :])
        return xt, wt

    # ---------- V projection: k-major waves of 4 chains, paced to DMA ----
    # bv65t is slotted into the middle of the x/w stream: it lands before
    # the first V-chain drain needs it, without delaying the early k-tiles.
    xtv = [xwA.tile([P, S], BF16, tag=f"ax{k}", name=f"axt{k}") for k in range(KT)]
    wtv = [xwA.tile([P, D], BF16, tag=f"aw{k}", name=f"awt{k}") for k in range(KT)]
    bv65t = const.tile([P, PAIRS * VW], BF16, tag="bv65t")
    for k in range(KT):
        if k == 5:
            nc.sync.dma_start(bv65t[:], io["bv65t"][:])
        nc.sync.dma_start(xtv[k][:], io["xvT"][k * P : (k + 1) * P, :])
        nc.sync.dma_start(wtv[k][:], io["wvT"][k * P : (k + 1) * P, :])
    bot = const.tile([P, D], BF16, tag="bot")
    nc.sync.dma_start(bot[:], io["bot"][:])
    xtq, wtq = load_xw(xwQ, io["xqT"], io["wqT"], "q")

    def emit_v_wave(ms):
        # software-pipelined: chain i runs one k-step behind chain i-1, so
        # its drain overlaps the other chains' remaining matmuls.
        tiles = [s_tile(f"vps{ms[0]}"), s_tile(f"vps{ms[1]}"),
                 c_tile(f"vps{ms[2]}"), c_tile(f"vps{ms[3]}")]
        for step in range(KT + len(ms) - 1):
            for i, (t, m) in enumerate(zip(tiles, ms)):
                k = step - i
                if not (0 <= k < KT):
                    continue
                for c in range(NCH):
                    nc.tensor.matmul(
                        t[:, c * NC : (c + 1) * NC],
                        xtv[k][:, m * P : (m + 1) * P],
                        wtv[k][:, c * NC : (c + 1) * NC],
                        start=(k == 0),
                        stop=(k == KT - 1),
                    )
                if k == KT - 1:
                    # scatter head pairs into V65 layout, fusing the bias add
                    psv = t.rearrange("p (pr two x) -> p pr two x", two=2, x=HD)
                    v = V65[m].rearrange("p (pr w) -> p pr w", w=VW)
                    bvv = bv65t.rearrange("p (pr w) -> p pr w", w=VW)
                    nc.vector.tensor_add(
                        v[:, :, 0:HD], psv[:, :, 0, :], bvv[:, :, 0:HD])
                    nc.vector.tensor_add(
                        v[:, :, 96:VW], psv[:, :, 1, :], bvv[:, :, 96:VW])

    emit_v_wave([0, 1, 2, 3])
    emit_v_wave([4, 5, 6, 7])

    # ---------- K projection: same wave structure (reuses V's SBUF slots) --
    xtk, wtk = load_xw(xwA, io["xkT"], io["wkT"], "a")
    wo_t = [wop.tile([P, D], BF16, tag=f"wo{k}", name=f"wo{k}") for k in range(KT)]
    for k in range(KT):
        nc.sync.dma_start(wo_t[k][:], io["woT"][k * P : (k + 1) * P, :])

    def emit_k_wave(ms):
        tiles = [s_tile(f"kps{ms[0]}"), s_tile(f"kps{ms[1]}"),
                 c_tile(f"kps{ms[2]}"), c_tile(f"kps{ms[3]}")]
        for step in range(KT + len(ms) - 1):
            for i, (t, m) in enumerate(zip(tiles, ms)):
                k = step - i
                if not (0 <= k < KT):
                    continue
                for c in range(NCH):
                    nc.tensor.matmul(
                        t[:, c * NC : (c + 1) * NC],
                        wtk[k][:, m * P : (m + 1) * P],
                        xtk[k][:, c * NC : (c + 1) * NC],
                        start=(k == 0),
                        stop=(k == KT - 1),
                    )
                if k == KT - 1:
                    # bias-add on ACT (Identity w/ per-partition bias): keeps
                    # the congested DVE queue out of the KT/QT ready path
                    nc.scalar.activation(
                        KTt[m][:], t[:], AF.Identity, bias=bkc[:, m : m + 1])

    emit_k_wave([0, 1, 2, 3])
    # Q(0) chain slots between the K waves so QT[0] is drained well before
    # scores(0,0); K wave 2 keeps the PE busy over the tensor_scalar_add.
    k_wave2 = lambda: emit_k_wave([4, 5, 6, 7])

    # ---------- attention ----------
    # persistent recip scratch: recip_approx_fast needs a full-partition
    # base-0 AP; only rows 64/32 are consumed — fill unused lanes once.
    rdn = const.tile([P, NC], F32, tag="denom")
    nc.vector.memset(rdn[:], 1.0)

    # Filler generators: each yields 4-matmul groups so score j-tiles can be
    # interleaved between them (keeps the PE fed while ACT drains exp).
    def qchain_groups(m):
        t = c_tile(f"qps{m}")
        for kk in range(0, KT, 2):
            def grp(kk=kk, t=t, m=m, last=(kk == KT - 2)):
                for k in (kk, kk + 1):
                    for c in range(NCH):
                        nc.tensor.matmul(
                            t[:, c * NC : (c + 1) * NC],
                            wtq[k][:, m * P : (m + 1) * P],
                            xtq[k][:, c * NC : (c + 1) * NC],
                            start=(k == 0),
                            stop=(k == KT - 1),
                        )
                if last:
                    nc.scalar.activation(
                        QT[m][:], t[:], AF.Identity, bias=bqc[:, m : m + 1])
            yield grp

    def av_groups(p, c, expEO):
        """4 matmul groups; caller must also schedule the returned finisher
        (via av_finisher) a few filler slots later.

        cols 0:512 rows 0-63 = OT_even, row 64 = colsum_even;
        cols 512:1024 rows 64-127 = OT_odd, row 32 = colsum_odd.
        The last group stages denominators and reciprocals on the DVE; the
        finisher then broadcasts the recip rows into the av tile's dead
        row ranges with K=1 PE matmuls and does same-lane multiplies:
          bcastE -> av[0:64, 512:1024]   (O-half rows 0-63 are free)
          bcastO -> av[64:128, 0:512]    (E-half rows 64-127 dead after copy)
        """
        av = c_tile("av")
        rcf = nrm.tile([P, NC], F32, tag="rcf")
        av_groups.state[(p, c)] = (av, rcf)
        for jj in range(0, KT, 2):
            def grp(jj=jj, av=av, p=p, c=c, expEO=expEO, rcf=rcf,
                    last=(jj == KT - 2)):
                for jt in (jj, jj + 1):
                    nc.tensor.matmul(
                        av[:, 0:NC], V65[jt][:, p * VW : p * VW + P],
                        expEO[:, jt * 2 * NC : jt * 2 * NC + NC],
                        start=(jt == 0), stop=(jt == KT - 1),
                    )
                    nc.tensor.matmul(
                        av[:, NC : 2 * NC],
                        V65[jt][:, p * VW + 32 : p * VW + 32 + P],
                        expEO[:, jt * 2 * NC + NC : (jt + 1) * 2 * NC],
                        start=(jt == 0), stop=(jt == KT - 1),
                    )
                if last:
                    nc.vector.tensor_copy(
                        rdn[HD : HD + 1, :], av[HD : HD + 1, 0:NC])
                    nc.vector.tensor_copy(rdn[32:33, :], av[32:33, NC : 2 * NC])
                    nc.vector.reciprocal_approx_fast(rcf[:], rdn[:])
            yield grp

    av_groups.state = {}

    def av_finisher(p, c):
        # Row->partitions broadcast of the two recip rows runs on the DMA
        # engine (0-stride free-dim source AP); DVE then does same-lane muls
        # with one PSUM operand each.
        av, rcf = av_groups.state.pop((p, c))
        def fin(av=av, rcf=rcf, p=p, c=c):
            Rt = nrm.tile([P, NC], F32, tag="Rt")
            nc.sync.dma_start(Rt[0:HD, :], rep_row(rcf[HD : HD + 1, :], HD))
            nc.sync.dma_start(Rt[HD:P, :], rep_row(rcf[32:33, :], HD))
            nc.vector.tensor_mul(
                OT[p][0:HD, c * NC : (c + 1) * NC],
                av[0:HD, 0:NC], Rt[0:HD, :])
            nc.vector.tensor_mul(
                OT[p][HD:P, c * NC : (c + 1) * NC],
                av[HD:P, NC : 2 * NC], Rt[HD:P, :])
        return fin

    def op_groups(m, tile_fn=None):
        t = (tile_fn or c_tile)(f"ops{m}")
        for kk in range(0, KT, 2):
            def grp(kk=kk, t=t, m=m, last=(kk == KT - 2)):
                for kt in (kk, kk + 1):
                    for cd in range(NCH):
                        nc.tensor.matmul(
                            t[:, cd * NC : (cd + 1) * NC],
                            OT[kt][:, m * P : (m + 1) * P],
                            wo_t[kt][:, cd * NC : (cd + 1) * NC],
                            start=(kt == 0), stop=(kt == KT - 1),
                        )
                if last:
                    osb = osbp.tile([P, 2 * NC], BF16, tag="osb")
                    nc.vector.tensor_add(osb[:], t[:], bot[:])
                    nc.sync.dma_start(io["out"][m * P : (m + 1) * P, :], osb[:])
            yield grp

    def emit_scores(p, c, fillers):
        """exp(scores/8) for heads 2p/2p+1 into one [128, 8*1024] SBUF buf.

        One filler group runs after each score j-tile so the PE has queued
        work while the exp stream catches up.
        """
        expEO = expp.tile([P, KT * 2 * NC], BF16, tag="expEO")
        fi = 0
        for j in range(KT):
            s = s_tile("sEO")
            nc.tensor.matmul(
                s[:, 0:NC],
                KTt[p][0:HD, j * P : (j + 1) * P],
                QT[p][0:HD, c * NC : (c + 1) * NC],
                start=True, stop=True,
            )
            nc.tensor.matmul(
                s[:, NC : 2 * NC],
                KTt[p][HD:P, j * P : (j + 1) * P],
                QT[p][HD:P, c * NC : (c + 1) * NC],
                start=True, stop=True,
            )
            nc.scalar.activation(
                expEO[:, j * 2 * NC : (j + 1) * 2 * NC], s[:], AF.Exp,
                scale=0.125)
            if fi < len(fillers):
                fillers[fi]()
                fi += 1
        while fi < len(fillers):
            fillers[fi]()
            fi += 1
        return expEO

    # Q(0) runs whole before the stream starts, then K wave 2
    for g in qchain_groups(0):
        g()
    k_wave2()

    # c=0 sweep: iteration p computes scores(p,0) with Q(p+1), av(p-1,0)
    # and the previous AV's normalize finisher interleaved between the
    # score j-tiles. The finisher sits a few slots in so the DVE recip
    # chain has drained by the time its PE broadcasts issue.
    exp_tiles = {}
    pending_fin = None
    for p in range(PAIRS):
        qg = list(qchain_groups(p + 1)) if p + 1 < PAIRS else []
        avg = (list(av_groups(p - 1, 0, exp_tiles.pop((p - 1, 0))))
               if p > 0 else [])
        # the finisher has no PE work (DMA broadcast + DVE muls), so it is
        # bundled into a matmul filler slot instead of taking its own.
        fillers = list(qg)
        if pending_fin is not None and fillers:
            g3, fin = fillers[3], pending_fin
            # g3 first: its tensor_scalar_add (QT for the next iteration's
            # scores) must not queue behind the finisher's DMA-gated muls
            fillers[3] = lambda g3=g3, fin=fin: (g3(), fin())
        elif pending_fin is not None:
            fillers.append(pending_fin)
        fillers += avg
        pending_fin = av_finisher(p - 1, 0) if p > 0 else None
        exp_tiles[(p, 0)] = emit_scores(p, 0, fillers)

    # c=1 sweep: outproj chunks m=0..3 unlock once all c=0 AVs are done.
    # Their 16 matmul groups are spread round-robin over iterations 1-7 so
    # the PE never starves against the exp stream late in the sweep.
    avg = list(av_groups(PAIRS - 1, 0, exp_tiles.pop((PAIRS - 1, 0))))
    g2, fin = avg[2], pending_fin
    avg[2] = lambda g2=g2, fin=fin: (g2(), fin())
    exp_tiles[(0, 1)] = emit_scores(0, 1, avg)
    pending_fin = av_finisher(PAIRS - 1, 0)
    op_fill = []
    for m in range(4):
        op_fill += list(op_groups(m))
    op_share = [3, 3, 2, 2, 2, 2, 2]
    for p in range(1, PAIRS):
        avg = list(av_groups(p - 1, 1, exp_tiles.pop((p - 1, 1))))
        g2, fin = avg[2], pending_fin
        avg[2] = lambda g2=g2, fin=fin: (g2(), fin())
        take = op_share[p - 1]
        fillers = avg + op_fill[:take]
        op_fill = op_fill[take:]
        pending_fin = av_finisher(p - 1, 1)
        exp_tiles[(p, 1)] = emit_scores(p, 1, fillers)
    # tail: run the last AV, then outproj 4/5's kt<6 groups while its
    # normalize drains; the kt=6,7 groups (which read OT[7] c=1) come after.
    tail_av = list(av_groups(PAIRS - 1, 1, exp_tiles.pop((PAIRS - 1, 1))))
    tail_op4 = list(op_groups(4, tile_fn=s_tile))
    tail_op5 = list(op_groups(5, tile_fn=s_tile))
    tail_av[0]()
    pending_fin()
    for g in tail_av[1:]:
        g()
    av_finisher(PAIRS - 1, 1)()   # Rt DMAs issue as soon as recip lands
    for g in tail_op4[:3] + tail_op5[:3] + [tail_op4[3], tail_op5[3]]:
        g()
    for g in op_groups(6, tile_fn=s_tile):
        g()
    # final chunk: bias-add + store in halves so the last DVE/DMA work
    # starts before the chain's second half finishes; separate psum tiles
    # per half avoid any cd1-chain-vs-osb0 false dependency
    osb7 = osbp.tile([P, 2 * NC], BF16, tag="osb")
    for cd in range(NCH):
        t7 = s_tile(f"ops7{cd}")
        for kt in range(KT):
            nc.tensor.matmul(
                t7[:, 0:NC],
                OT[kt][:, 7 * P : 8 * P],
                wo_t[kt][:, cd * NC : (cd + 1) * NC],
                start=(kt == 0), stop=(kt == KT - 1),
            )
        nc.vector.tensor_add(
            osb7[:, cd * NC : (cd + 1) * NC], t7[:, 0:NC],
            bot[:, cd * NC : (cd + 1) * NC])
        nc.sync.dma_start(
            io["out"][7 * P : 8 * P, cd * NC : (cd + 1) * NC],
            osb7[:, cd * NC : (cd + 1) * NC])


def declare_io(nc):
    def din(name, shape, dt):
        return nc.dram_tensor(name, shape, dt, kind="ExternalInput").ap()

    io = {
        "xqT": din("xqT", [D, S], BF16),
        "xkT": din("xkT", [D, S], BF16),
        "xvT": din("xvT", [D, S], BF16),
        "wqT": din("wqT", [D, D], BF16),
        "wkT": din("wkT", [D, D], BF16),
        "wvT": din("wvT", [D, D], BF16),
        "woT": din("woT", [D, D], BF16),
        "bq_c": din("bq_c", [P, KT], F32),
        "bk_c": din("bk_c", [P, KT], F32),
        "bv65t": din("bv65t", [P, PAIRS * VW], BF16),
        "bot": din("bot", [P, D], BF16),
        "out": nc.dram_tensor("out", [S, D], BF16, kind="ExternalOutput").ap(),
    }
    return io


_NC_CACHE = {}


def get_nc():
    if "nc" not in _NC_CACHE:
        nc = bacc.Bacc(
            "TRN2",
            target_bir_lowering=False,
            debug=False,
            enable_asserts=False,
            num_devices=N_CORES,
        )
        io = declare_io(nc)
        with tile.TileContext(nc) as tc:
            with ExitStack() as ctx:
                build_body(ctx, tc, io)
        nc.compile()
        _NC_CACHE["nc"] = nc
    return _NC_CACHE["nc"]


def prep_inputs(query, key, value, Wq, bq, Wk, bk, Wv, bv, Wo, bo):
    bf = ml_dtypes.bfloat16
    f32 = np.float32

    def t16(a):
        return np.ascontiguousarray(np.asarray(a, dtype=f32).T).astype(bf)

    # bv in V65 pair layout: per pair [even-head dims | pad 32 | odd-head dims]
    bv65 = np.zeros((1, PAIRS * VW), dtype=f32)
    bvf = np.asarray(bv, dtype=f32)
    for pr in range(PAIRS):
        bv65[0, pr * VW : pr * VW + HD] = bvf[pr * 2 * HD : pr * 2 * HD + HD]
        bv65[0, pr * VW + 96 : pr * VW + VW] = bvf[pr * 2 * HD + HD : (pr + 1) * 2 * HD]
    bv65 = bv65.astype(bf)

    base = {
        "wqT": t16(Wq),
        "wkT": t16(Wk),
        "wvT": t16(Wv),
        "woT": t16(Wo),
        "bq_c": np.ascontiguousarray(
            np.asarray(bq, dtype=f32).reshape(KT, P).T),
        "bk_c": np.ascontiguousarray(
            np.asarray(bk, dtype=f32).reshape(KT, P).T),
        "bv65t": np.ascontiguousarray(np.broadcast_to(bv65, (P, PAIRS * VW))),
        "bot": np.ascontiguousarray(np.broadcast_to(
            np.asarray(bo, dtype=f32).astype(bf).reshape(1, D), (P, D))),
    }
    in_maps = []
    for b in range(np.asarray(query).shape[0]):
        m = dict(base)
        m["xqT"] = t16(query[b])
        m["xkT"] = t16(key[b])
        m["xvT"] = t16(value[b])
        in_maps.append(m)
    return in_maps


def kernel(query, key, value, Wq, bq, Wk, bk, Wv, bv, Wo, bo, **run_kwargs):
    nc = get_nc()
    in_maps = prep_inputs(query, key, value, Wq, bq, Wk, bk, Wv, bv, Wo, bo)
    res = run_bass_kernel_spmd(
        nc, in_maps, core_ids=list(range(N_CORES)), **run_kwargs)
    out = np.stack(
        [res.results[b]["out"] for b in range(N_CORES)], axis=0
    ).astype(np.float32)
    if run_kwargs:
        kernel.last_results = res
    return out
